# revision 22
# baseline (speedup 1.0000x reference)
"""Trainium2 Bass kernel for nn_DeformableBlock (deformable attention block).

Algorithm (per core = one batch element, data-parallel over batch):
  1. PE: femb[l] = feat_l^T @ embed_w[l]  (project feature maps once, 32-dim),
     stored to DRAM in bf16 as 4-corner rows
       femb4[r] = [femb[r], femb[r+1], femb[r+W], femb[r+W+1]]  (128 bf16 = 256B)
     so ONE 256B dma_gather descriptor fetches all 4 bilinear corners.
  2. PE: per 128-query block, transpose x tile and compute attn/offset logits.
  3. DVE/ACT: softmax over samples, tanh offsets, positions, floor via the
     RNE magic-constant trick, per-corner weights with zero-padding edge
     logic folded in, flat int16 indices.
  4. DMA: partition-fold indices into dma_gather's wrapped [16, N/16] layout,
     then ONE dma_gather per 128-query block (4096 descriptors, elem 128 bf16).
  5. ACT: expand per-(sample,corner) coefs across the 32 head-dims (bf16).
     DVE: stride-1 bf16 multiply + corner/sample reduction tree (2x mode).
"""

import sys

for _p in ("/opt/trn_rl_repo",):
    if _p not in sys.path:
        sys.path.insert(0, _p)

import numpy as np
from contextlib import ExitStack

import concourse.bass as bass
import concourse.bacc as bacc
import concourse.tile as tile
from concourse import mybir
from concourse.bass import AP
from concourse.bass_utils import run_bass_kernel_spmd
from concourse.masks import make_identity

F32 = mybir.dt.float32
BF16 = mybir.dt.bfloat16
I16 = mybir.dt.int16
AF = mybir.ActivationFunctionType
OP = mybir.AluOpType

B, L, P, C = 8, 4, 1024, 256
NH, NS, HD = 8, 4, 32
LEVEL_HW = [(64, 64), (32, 32), (16, 16), (8, 8)]
NQ = L * P          # queries per core
QB = NQ // 128      # 32 query blocks of 128
BPL = QB // L       # 8 blocks per level
RNE_M = 12582912.0  # 1.5*2^23; f+M lands in [2^23,2^24) where ulp==1


def _ap(t, offset, dims):
    """Raw AP on a DRAM tensor: offset and strides in flat elements."""
    return AP(tensor=t.tensor if isinstance(t, AP) else t, offset=offset,
              ap=[list(d) for d in dims])


def sv(t: AP, off: int, dims):
    """Strided free-dim view of an SBUF tile: keeps the partition dim,
    offsets `off` elements into each partition's free space."""
    base = t[:] if not isinstance(t, AP) else t
    pstride, nparts = base.ap[0]
    return AP(tensor=base.tensor, offset=base.offset + off,
              ap=[[pstride, nparts]] + [list(d) for d in dims])


def emit_kernel(ctx: ExitStack, tc: tile.TileContext, io: dict):
    nc = tc.nc
    x, ref = io["x"], io["ref"]
    feats = [io[f"feat{i}"] for i in range(L)]
    w_attn, b_attn = io["w_attn"], io["b_attn"]
    w_off, b_off = io["w_off"], io["b_off"]
    embed_w, embed_b = io["embed_w"], io["embed_b"]
    out = io["out"]
    femb4 = io["femb4"]  # 4 dram scratch tensors [HW, 128] bf16
    t2dL = io["t2d"]     # 4 dram scratch tensors [128*256] i16 (idx bounce)

    keep = ctx.enter_context(tc.tile_pool(name="keep", bufs=1))

    # ---- long-lived constants ----
    ident = keep.tile([128, 128], F32)
    make_identity(nc, ident)
    wcat = keep.tile([128, 2, 96], F32)  # k-halves of [w_attn | w_off]
    for k in range(2):
        nc.sync.dma_start(out=wcat[:, k, 0:32], in_=w_attn[k * 128:(k + 1) * 128, :])
        nc.sync.dma_start(out=wcat[:, k, 32:96], in_=w_off[k * 128:(k + 1) * 128, :])
    bias96 = keep.tile([128, 96], F32)
    nc.sync.dma_start(out=bias96[:, 0:32], in_=_ap(b_attn, 0, [[0, 128], [1, 32]]))
    nc.sync.dma_start(out=bias96[:, 32:96], in_=_ap(b_off, 0, [[0, 128], [1, 64]]))
    ebt = keep.tile([128, L, HD], F32)
    nc.sync.dma_start(out=ebt[:], in_=_ap(embed_b, 0, [[0, 128], [1, L * HD]]))
    # per-level tiles so phase-3 readers only depend on their own level's
    # writes (coarse tile-granularity dependency tracking otherwise stalls
    # the first gather on the whole front-end)
    c4L = [keep.tile([128, BPL * 128], F32, name=f"c4_{i}", tag=f"c4_{i}") for i in range(L)]
    idxwL = [keep.tile([128, BPL * 256], I16, name=f"idxw_{i}", tag=f"idxw_{i}") for i in range(L)]
    zsrc = keep.tile([128, 128], BF16)
    nc.vector.memset(zsrc[:], 0.0)
    permP = keep.tile([128, 128], F32)
    nc.sync.dma_start(out=permP[:], in_=io["permP"][:])
    lgL = [keep.tile([128, BPL, 96], F32, name=f"lg_{i}", tag=f"lg_{i}") for i in range(L)]
    refc = keep.tile([128, QB * 2], F32)
    nc.sync.dma_start(out=refc[:], in_=_ap(ref, 0, [[2, 128], [256, QB], [1, 2]]))
    ps = ctx.enter_context(tc.tile_pool(name="ps", bufs=3, space="PSUM"))
    ps2 = ctx.enter_context(tc.tile_pool(name="ps2", bufs=2, space="PSUM"))

    # ======== software-pipelined phases (levels processed smallest-first so
    # the first gathers start early): femb all levels -> logits/prep/idx all
    # levels -> gather+combine all blocks ====
    LV_ORDER = [3, 2, 1, 0]
    with ExitStack() as p1:
        fpool = p1.enter_context(tc.tile_pool(name="fpool", bufs=1))
        fsm = p1.enter_context(tc.tile_pool(name="fsm", bufs=2))
        blockio = p1.enter_context(tc.tile_pool(name="blockio", bufs=4))
        xpool = p1.enter_context(tc.tile_pool(name="xpool", bufs=2))
        prep = p1.enter_context(tc.tile_pool(name="prep", bufs=1))
        gpool = p1.enter_context(tc.tile_pool(name="gpool", bufs=4))
        cxp = p1.enter_context(tc.tile_pool(name="cxp", bufs=2))
        tpool = p1.enter_context(tc.tile_pool(name="tpool", bufs=2))
        opool = p1.enter_context(tc.tile_pool(name="opool", bufs=2))
        xf = x.rearrange("l p c -> (l p) c")

        # ---- stage all feature maps (pad zeroed: shifted femb windows
        # read past each level's slice; the results are never gathered) ----
        HWTOT = sum(h * w for h, w in LEVEL_HW)
        PAD = 80
        fsb = fpool.tile([128, 2, HWTOT + PAD], F32, tag="feat")
        nc.vector.memset(fsb[:, :, HWTOT:], 0.0)
        foff = {}
        off = 0
        for lv in LV_ORDER:
            H, W = LEVEL_HW[lv]
            foff[lv] = off
            fl = feats[lv].rearrange("c h w -> c (h w)")
            for k in range(2):
                nc.sync.dma_start(out=fsb[:, k, off:off + H * W],
                                  in_=fl[k * 128:(k + 1) * 128, :])
            off += H * W

        def emit_femb(lv):
            """femb4[r] = [femb[r], femb[r+1], femb[r+W], femb[r+W+1]] built
            directly on the PE via free-dim-shifted lhsT windows, so the DRAM
            write is contiguous (cheap HWDGE descriptors). Issued on the
            scalar engine's DGE ring right after its ACT psum copy."""
            H, W = LEVEL_HW[lv]
            HW = H * W
            MT = (HW + 127) // 128
            ew = fsm.tile([128, 2, HD], F32, tag="ew", name="ew")
            nc.sync.dma_start(
                out=ew[:],
                in_=_ap(embed_w, lv * 256 * HD,
                        [[HD, 128], [128 * HD, 2], [1, HD]]),
            )
            f4 = femb4[lv]
            for m0 in range(0, MT, 8):
                mc = min(8, MT - m0)
                fe4 = fsm.tile([128, 8, 128], BF16, tag="fe", name="fe4")
                mp = 128
                for m in range(m0, m0 + mc):
                    mp = min(128, HW - m * 128)
                    ps4 = ps2.tile([128, 4, HD], F32, tag="psA", name="ps4")
                    for ci, dr in enumerate((0, 1, W, W + 1)):
                        w0 = foff[lv] + m * 128 + dr
                        for k in range(2):
                            nc.tensor.matmul(
                                ps4[:mp, ci, :], lhsT=fsb[:, k, w0:w0 + mp],
                                rhs=ew[:, k, :], start=(k == 0), stop=(k == 1),
                            )
                    nc.scalar.copy(fe4[:mp, m - m0, :], ps4[:mp, :, :])
                nc.scalar.dma_start(
                    out=_ap(f4, m0 * 128 * 128,
                            [[128, mp], [128 * 128, mc], [1, 128]]),
                    in_=fe4[:mp, 0:mc, :],
                )

        def emit_logits(lv):
            # one-block software pipeline: the PE issues block g+1's
            # transposes while the ACT engine evacuates block g's xt, so the
            # PE never stalls on the cross-engine round-trip
            g0 = lv * BPL
            xqL = xpool.tile([128, BPL, 256], F32, tag="xq", name="xqL")
            nc.sync.dma_start(
                out=xqL[:],
                in_=_ap(x, lv * P * C, [[256, 128], [128 * 256, BPL], [1, 256]]),
            )
            pend = []

            def flush():
                gp_, xtp = pend.pop(0)
                lg = ps2.tile([128, 96], F32, tag="plg", name="lg")
                for k in range(2):
                    nc.tensor.matmul(lg[:], lhsT=xtp[:, k, :],
                                     rhs=wcat[:, k, :],
                                     start=(k == 0), stop=(k == 1))
                nc.scalar.copy(lgL[lv][:, gp_ - g0, :], lg[:])

            for g in range(g0, g0 + BPL):
                xt = blockio.tile([128, 2, 128], F32, tag="xt", name="xt")
                for k in range(2):
                    pt_ = ps.tile([128, 128], F32, tag="ptr", name="pt_")
                    nc.tensor.transpose(
                        pt_[:], xqL[:, g - g0, k * 128:(k + 1) * 128], ident[:])
                    nc.scalar.copy(xt[:, k, :], pt_[:])
                pend.append((g, xt))
                if len(pend) >= 2:
                    flush()
            while pend:
                flush()
            nc.vector.tensor_add(
                lgL[lv][:], lgL[lv][:],
                sv(bias96, 0, [[0, BPL], [1, 96]]))

        def emit_prep(lv):
            H, W = LEVEL_HW[lv]
            g0 = lv * BPL
            kap = 0.5 * (W - 1)
            ea = prep.tile([128, 256], F32, tag="ea", name="ea")
            nc.scalar.activation(
                ea[:], sv(lgL[lv], 0, [[96, BPL], [1, 32]]), AF.Exp)
            s2 = prep.tile([128, 128], F32, tag="s2", name="s2")
            nc.vector.tensor_add(s2[:], sv(ea, 0, [[4, 64], [1, 2]]),
                                 sv(ea, 2, [[4, 64], [1, 2]]))
            s1 = prep.tile([128, 64], F32, tag="s1", name="s1")
            nc.vector.tensor_add(s1[:], sv(s2, 0, [[2, 64]]),
                                 sv(s2, 1, [[2, 64]]))
            dinv = prep.tile([128, 64], F32, tag="dinv", name="dinv")
            nc.vector.reciprocal(dinv[:], s1[:])
            a_h = prep.tile([128, 256], F32, tag="a_h", name="a_h")
            nc.vector.tensor_mul(a_h[:], ea[:],
                                 sv(dinv, 0, [[1, 64], [0, 4]]))

            T1 = prep.tile([128, 512], F32, tag="T1", name="T1")
            nc.scalar.activation(
                T1[:], sv(lgL[lv], 32, [[96, BPL], [1, 64]]), AF.Tanh)
            nc.vector.tensor_add(T1[:], T1[:],
                                 sv(refc, g0 * 2, [[2, BPL], [0, 32], [1, 2]]))
            nc.vector.tensor_scalar(T1[:], T1[:], kap, kap, OP.mult, OP.add)
            # floor via int16 cast round-trip (all-DVE; the is_gt fix below
            # corrects any round-up, so truncate vs RNE both work)
            Tfi = prep.tile([128, 512], I16, tag="Tfi", name="Tfi")
            nc.vector.tensor_copy(Tfi[:], T1[:])
            T2 = prep.tile([128, 512], F32, tag="T2", name="T2")
            nc.vector.tensor_copy(T2[:], Tfi[:])
            T3 = prep.tile([128, 512], F32, tag="T3", name="T3")
            nc.vector.tensor_tensor(T3[:], T2[:], T1[:], OP.is_gt)
            nc.vector.tensor_tensor(T2[:], T2[:], T3[:], OP.subtract)   # x0f
            nc.vector.tensor_tensor(T3[:], T1[:], T2[:], OP.subtract)   # w1f
            nc.vector.tensor_scalar(T1[:], T3[:], -1.0, 1.0, OP.mult, OP.add)
            T4 = prep.tile([128, 512], F32, tag="T4", name="T4")  # xb
            nc.vector.tensor_scalar(T4[:], T2[:], 0.0, float(W - 2),
                                    OP.max, OP.min)
            nc.vector.tensor_tensor(T2[:], T2[:], T4[:], OP.subtract)   # d
            T5 = prep.tile([128, 512], F32, tag="T5", name="T5")  # e0 -> wB
            nc.vector.tensor_scalar(T5[:], T2[:], 0.0, None, OP.is_equal)
            T6 = prep.tile([128, 512], F32, tag="T6", name="T6")  # em1
            nc.vector.tensor_scalar(T6[:], T2[:], -1.0, None, OP.is_equal)
            nc.vector.tensor_scalar(T2[:], T2[:], 1.0, None, OP.is_equal)
            T7 = prep.tile([128, 512], F32, tag="T7", name="T7")  # wA
            nc.vector.tensor_tensor(T7[:], T1[:], T5[:], OP.mult)
            nc.vector.tensor_tensor(T6[:], T3[:], T6[:], OP.mult)
            nc.vector.tensor_add(T7[:], T7[:], T6[:])
            nc.vector.tensor_tensor(T5[:], T3[:], T5[:], OP.mult)
            nc.vector.tensor_tensor(T2[:], T1[:], T2[:], OP.mult)
            nc.vector.tensor_add(T5[:], T5[:], T2[:])

            fly = prep.tile([128, 256], F32, tag="fly", name="fly")
            nc.vector.tensor_scalar_mul(fly[:], sv(T4, 1, [[2, 256]]), float(W))
            nc.vector.tensor_add(fly[:], fly[:], sv(T4, 0, [[2, 256]]))
            T2i = prep.tile([128, 2, 128], I16, tag="T2i", name="T2i")
            for j in range(2):
                pf = ps.tile([128, 128], F32, tag="ptr", name="pf")
                nc.tensor.matmul(pf[:], lhsT=fly[:, j * 128:(j + 1) * 128],
                                 rhs=permP[:], start=True, stop=True)
                nc.vector.tensor_copy(T2i[:, j, :], pf[:])
            # bounce T2i through DRAM to land the wrapped index layout in 3
            # DMAs; gathers only use queues {0,1} so replication stops at
            # partition 64
            t2d = t2dL[lv]
            nc.sync.dma_start(out=_ap(t2d, 0, [[256, 128], [1, 256]]),
                              in_=T2i[:])
            for j in range(2):
                nc.sync.dma_start(
                    out=idxwL[lv][0:16, j * 1024:(j + 1) * 1024],
                    in_=_ap(t2d, j * 128, [[8, 16], [256, 128], [1, 8]]),
                )
            for t in range(1, 4):
                nc.sync.dma_start(
                    out=idxwL[lv][t * 16:(t + 1) * 16, :],
                    in_=idxwL[lv][0:16, :])

            # per-corner coefs; corner index = yi*2 + si to match the
            # [r, r+1, r+W, r+W+1] row layout of femb4.
            wxa = prep.tile([128, 256], F32, tag="wxa", name="wxa")
            nc.vector.tensor_mul(wxa[:], sv(T7, 0, [[2, 256]]), a_h[:])
            wxb = prep.tile([128, 256], F32, tag="wxb", name="wxb")
            nc.vector.tensor_mul(wxb[:], sv(T5, 0, [[2, 256]]), a_h[:])
            for si, wx in ((0, wxa), (1, wxb)):
                for yi, wy in ((0, T7), (1, T5)):
                    nc.vector.tensor_mul(
                        sv(c4L[lv], yi * 2 + si, [[4, 256]]),
                        wx[:],
                        sv(wy, 1, [[2, 256]]),
                    )

        # interleaved emission: each level's prep/T2i lands on the PE right
        # after that level's logits so its gathers unblock early; the heavy
        # femb matmul batches for the big levels go last (their gathers are
        # hundreds of us out)
        emit_logits(3)
        emit_prep(3)
        emit_femb(3)
        emit_femb(2)
        emit_logits(2)
        emit_prep(2)
        emit_logits(1)
        emit_prep(1)
        emit_femb(1)
        emit_logits(0)
        emit_prep(0)
        emit_femb(0)

        # ---- phase 3: gather + combine for all blocks ----
        for lv in LV_ORDER:
            H, W = LEVEL_HW[lv]
            HW = H * W
            g0 = lv * BPL
            for g in range(g0, g0 + BPL):
                gi = g - g0
                # expand coefs over the 32 head-dims on the Scalar engine
                c4x = cxp.tile([128, 4096], BF16, tag="c4x")
                nc.scalar.copy(
                    c4x[:],
                    sv(c4L[lv], gi * 128, [[4, 32], [1, 4], [0, 32]]),
                )
                gb = gpool.tile([128, 32, 128], BF16, tag="gb")
                for c in range(4):
                    nc.gpsimd.dma_gather(
                        gb[:, c * 8:(c + 1) * 8, :],
                        _ap(femb4[lv], 0, [[128, HW], [1, 128]]),
                        idxwL[lv][:, gi * 256 + c * 64:gi * 256 + (c + 1) * 64],
                        1024,
                        1024,
                        128,
                        elem_step=128,
                        queue_num=c % 2,
                    )
                nc.vector.tensor_mul(gb[:], gb[:], c4x[:])
                t1 = tpool.tile([128, 2048], BF16, tag="t1")
                nc.vector.tensor_add(
                    t1[:],
                    sv(gb, 0, [[128, 32], [64, 2], [1, 32]]),
                    sv(gb, 32, [[128, 32], [64, 2], [1, 32]]),
                )
                t2 = tpool.tile([128, 1024], BF16, tag="t2")
                nc.vector.tensor_add(
                    t2[:],
                    sv(t1, 0, [[64, 32], [1, 32]]),
                    sv(t1, 32, [[64, 32], [1, 32]]),
                )
                t3 = tpool.tile([128, 512], BF16, tag="t3")
                nc.vector.tensor_add(
                    t3[:],
                    sv(t2, 0, [[128, 8], [1, 64]]),
                    sv(t2, 64, [[128, 8], [1, 64]]),
                )
                if gi % 4 == 0:
                    ob4 = opool.tile([128, 4, 256], F32, tag="ob", name="ob4")
                ob = ob4[:, gi % 4, :]
                nc.vector.tensor_add(
                    ob,
                    sv(t3, 0, [[64, 8], [1, 32]]),
                    sv(t3, 32, [[64, 8], [1, 32]]),
                )
                nc.vector.tensor_add(ob, ob,
                                     sv(ebt, lv * HD, [[0, 8], [1, 32]]))
                if gi % 4 == 3:
                    nc.sync.dma_start(
                        out=_ap(out, (g - 3) * 128 * 256,
                                [[256, 128], [128 * 256, 4], [1, 256]]),
                        in_=ob4[:],
                    )


def build_program():
    nc = bacc.Bacc("TRN2", target_bir_lowering=False, debug=False,
                   num_swdge_queues=4)
    io = {}
    io["x"] = nc.dram_tensor("x", [L, P, C], F32, kind="ExternalInput").ap()
    io["ref"] = nc.dram_tensor("ref", [L, P, 2], F32, kind="ExternalInput").ap()
    for i, (H, W) in enumerate(LEVEL_HW):
        io[f"feat{i}"] = nc.dram_tensor(f"feat{i}", [C, H, W], F32,
                                        kind="ExternalInput").ap()
    io["w_attn"] = nc.dram_tensor("w_attn", [C, NH * NS], F32,
                                  kind="ExternalInput").ap()
    io["b_attn"] = nc.dram_tensor("b_attn", [NH * NS], F32,
                                  kind="ExternalInput").ap()
    io["w_off"] = nc.dram_tensor("w_off", [C, 2 * NH * NS], F32,
                                 kind="ExternalInput").ap()
    io["b_off"] = nc.dram_tensor("b_off", [2 * NH * NS], F32,
                                 kind="ExternalInput").ap()
    io["embed_w"] = nc.dram_tensor("embed_w", [L, C, HD], F32,
                                   kind="ExternalInput").ap()
    io["embed_b"] = nc.dram_tensor("embed_b", [L, HD], F32,
                                   kind="ExternalInput").ap()
    io["permP"] = nc.dram_tensor("permP", [128, 128], F32,
                                 kind="ExternalInput").ap()
    io["out"] = nc.dram_tensor("out", [L, P, NH * HD], F32,
                               kind="ExternalOutput").ap()
    io["femb4"] = [
        nc.dram_tensor(f"femb4_{i}", [H * W, 128], BF16, kind="Internal").ap()
        for i, (H, W) in enumerate(LEVEL_HW)
    ]
    io["t2d"] = [
        nc.dram_tensor(f"t2d_{i}", [128 * 256], I16, kind="Internal").ap()
        for i in range(L)
    ]
    with tile.TileContext(nc) as tc:
        with ExitStack() as ctx:
            emit_kernel(ctx, tc, io)
    nc.compile()
    return nc


_prog = None


def kernel(**inputs):
    global _prog
    if _prog is None:
        _prog = build_program()
    nc = _prog
    res = run_bass_kernel_spmd(nc, _in_maps(inputs), list(range(B)))
    out = np.stack([res.results[i]["out"] for i in range(B)], axis=0)
    return out.reshape(B, L, P, NH * HD)


def _perm_matrix():
    p = np.zeros((128, 128), np.float32)
    for n in range(128):
        p[(n % 8) * 16 + n // 8, n] = 1.0
    return p


def _in_maps(inputs):
    keys = ["x", "ref", "feat0", "feat1", "feat2", "feat3",
            "w_attn", "b_attn", "w_off", "b_off", "embed_w", "embed_b"]
    per_batch = {"x", "ref", "feat0", "feat1", "feat2", "feat3"}
    pm = _perm_matrix()
    maps = []
    for b in range(B):
        m = {"permP": pm}
        for kk in keys:
            v = np.ascontiguousarray(np.asarray(inputs[kk], dtype=np.float32))
            m[kk] = v[b] if kk in per_batch else v
        maps.append(m)
    return maps


def profile(inputs):
    """Run with tracing; returns HW exec time in ns (or None if unavailable)."""
    global _prog
    if _prog is None:
        _prog = build_program()
    res = run_bass_kernel_spmd(_prog, _in_maps(inputs), list(range(B)), trace=True)
    return res.exec_time_ns


if __name__ == "__main__":
    build_program()
    print("build ok")


# revision 23
# speedup vs baseline: 1.4451x; 1.4451x over previous
"""Trainium2 Bass kernel for nn_DeformableBlock (deformable attention block).

Algorithm (per core = one batch element, data-parallel over batch):
  1. PE: femb[l] = feat_l^T @ embed_w[l]  (project feature maps once, 32-dim),
     stored to DRAM in bf16 as 4-corner rows
       femb4[r] = [femb[r], femb[r+1], femb[r+W], femb[r+W+1]]  (128 bf16 = 256B)
     so ONE 256B dma_gather descriptor fetches all 4 bilinear corners.
  2. PE: per 128-query block, transpose x tile and compute attn/offset logits.
  3. DVE/ACT: softmax over samples, tanh offsets, positions, floor via the
     RNE magic-constant trick, per-corner weights with zero-padding edge
     logic folded in, flat int16 indices.
  4. DMA: partition-fold indices into dma_gather's wrapped [16, N/16] layout,
     then ONE dma_gather per 128-query block (4096 descriptors, elem 128 bf16).
  5. ACT: expand per-(sample,corner) coefs across the 32 head-dims (bf16).
     DVE: stride-1 bf16 multiply + corner/sample reduction tree (2x mode).
"""

import sys

for _p in ("/opt/trn_rl_repo",):
    if _p not in sys.path:
        sys.path.insert(0, _p)

import numpy as np
from contextlib import ExitStack

import concourse.bass as bass
import concourse.bacc as bacc
import concourse.tile as tile
from concourse import mybir
from concourse.bass import AP
from concourse.bass_utils import run_bass_kernel_spmd
from concourse.masks import make_identity

F32 = mybir.dt.float32
BF16 = mybir.dt.bfloat16
I16 = mybir.dt.int16
AF = mybir.ActivationFunctionType
OP = mybir.AluOpType

B, L, P, C = 8, 4, 1024, 256
NH, NS, HD = 8, 4, 32
LEVEL_HW = [(64, 64), (32, 32), (16, 16), (8, 8)]
NQ = L * P          # queries per core
QB = NQ // 128      # 32 query blocks of 128
BPL = QB // L       # 8 blocks per level
RNE_M = 12582912.0  # 1.5*2^23; f+M lands in [2^23,2^24) where ulp==1


def _ap(t, offset, dims):
    """Raw AP on a DRAM tensor: offset and strides in flat elements."""
    return AP(tensor=t.tensor if isinstance(t, AP) else t, offset=offset,
              ap=[list(d) for d in dims])


def sv(t: AP, off: int, dims):
    """Strided free-dim view of an SBUF tile: keeps the partition dim,
    offsets `off` elements into each partition's free space."""
    base = t[:] if not isinstance(t, AP) else t
    pstride, nparts = base.ap[0]
    return AP(tensor=base.tensor, offset=base.offset + off,
              ap=[[pstride, nparts]] + [list(d) for d in dims])


def emit_kernel(ctx: ExitStack, tc: tile.TileContext, io: dict):
    nc = tc.nc
    x, ref = io["x"], io["ref"]
    feats = [io[f"feat{i}"] for i in range(L)]
    w_attn, b_attn = io["w_attn"], io["b_attn"]
    w_off, b_off = io["w_off"], io["b_off"]
    embed_w, embed_b = io["embed_w"], io["embed_b"]
    out = io["out"]
    femb4 = io["femb4"]  # 4 dram scratch tensors [HW, 128] bf16
    t2dL = io["t2d"]     # 4 dram scratch tensors [128*256] i16 (idx bounce)

    keep = ctx.enter_context(tc.tile_pool(name="keep", bufs=1))

    # ---- long-lived constants ----
    ident = keep.tile([128, 128], F32)
    make_identity(nc, ident)
    wcat = keep.tile([128, 2, 96], F32)  # k-halves of [w_attn | w_off]
    for k in range(2):
        nc.sync.dma_start(out=wcat[:, k, 0:32], in_=w_attn[k * 128:(k + 1) * 128, :])
        nc.sync.dma_start(out=wcat[:, k, 32:96], in_=w_off[k * 128:(k + 1) * 128, :])
    bias96 = keep.tile([128, 96], F32)
    nc.sync.dma_start(out=bias96[:, 0:32], in_=_ap(b_attn, 0, [[0, 128], [1, 32]]))
    nc.sync.dma_start(out=bias96[:, 32:96], in_=_ap(b_off, 0, [[0, 128], [1, 64]]))
    ebt = keep.tile([128, L, HD], F32)
    nc.sync.dma_start(out=ebt[:], in_=_ap(embed_b, 0, [[0, 128], [1, L * HD]]))
    # per-level tiles so phase-3 readers only depend on their own level's
    # writes (coarse tile-granularity dependency tracking otherwise stalls
    # the first gather on the whole front-end)
    c4L = [keep.tile([128, BPL * 128], F32, name=f"c4_{i}", tag=f"c4_{i}") for i in range(L)]
    idxwL = [keep.tile([128, BPL * 256], I16, name=f"idxw_{i}", tag=f"idxw_{i}") for i in range(L)]
    zsrc = keep.tile([128, 128], BF16)
    nc.vector.memset(zsrc[:], 0.0)
    permP = keep.tile([128, 128], F32)
    nc.sync.dma_start(out=permP[:], in_=io["permP"][:])
    lgL = [keep.tile([128, BPL, 96], F32, name=f"lg_{i}", tag=f"lg_{i}") for i in range(L)]
    refc = keep.tile([128, QB * 2], F32)
    nc.sync.dma_start(out=refc[:], in_=_ap(ref, 0, [[2, 128], [256, QB], [1, 2]]))
    ps = ctx.enter_context(tc.tile_pool(name="ps", bufs=3, space="PSUM"))
    ps2 = ctx.enter_context(tc.tile_pool(name="ps2", bufs=2, space="PSUM"))

    # ======== software-pipelined phases (levels processed smallest-first so
    # the first gathers start early): femb all levels -> logits/prep/idx all
    # levels -> gather+combine all blocks ====
    LV_ORDER = [3, 2, 1, 0]
    with ExitStack() as p1:
        fpool = p1.enter_context(tc.tile_pool(name="fpool", bufs=1))
        fsm = p1.enter_context(tc.tile_pool(name="fsm", bufs=2))
        blockio = p1.enter_context(tc.tile_pool(name="blockio", bufs=4))
        xpool = p1.enter_context(tc.tile_pool(name="xpool", bufs=2))
        prep = p1.enter_context(tc.tile_pool(name="prep", bufs=1))
        gpool = p1.enter_context(tc.tile_pool(name="gpool", bufs=4))
        cxp = p1.enter_context(tc.tile_pool(name="cxp", bufs=2))
        tpool = p1.enter_context(tc.tile_pool(name="tpool", bufs=2))
        opool = p1.enter_context(tc.tile_pool(name="opool", bufs=2))
        xf = x.rearrange("l p c -> (l p) c")

        # ---- stage all feature maps (pad zeroed: shifted femb windows
        # read past each level's slice; the results are never gathered) ----
        HWTOT = sum(h * w for h, w in LEVEL_HW)
        PAD = 80
        fsb = fpool.tile([128, 2, HWTOT + PAD], F32, tag="feat")
        nc.vector.memset(fsb[:, :, HWTOT:], 0.0)
        foff = {}
        off = 0
        for lv in LV_ORDER:
            H, W = LEVEL_HW[lv]
            foff[lv] = off
            fl = feats[lv].rearrange("c h w -> c (h w)")
            for k in range(2):
                nc.sync.dma_start(out=fsb[:, k, off:off + H * W],
                                  in_=fl[k * 128:(k + 1) * 128, :])
            off += H * W

        def emit_femb(lv):
            """femb4[r] = [femb[r], femb[r+1], femb[r+W], femb[r+W+1]] built
            directly on the PE via free-dim-shifted lhsT windows, so the DRAM
            write is contiguous (cheap HWDGE descriptors). Issued on the
            scalar engine's DGE ring right after its ACT psum copy."""
            H, W = LEVEL_HW[lv]
            HW = H * W
            MT = (HW + 127) // 128
            ew = fsm.tile([128, 2, HD], F32, tag="ew", name="ew")
            nc.sync.dma_start(
                out=ew[:],
                in_=_ap(embed_w, lv * 256 * HD,
                        [[HD, 128], [128 * HD, 2], [1, HD]]),
            )
            f4 = femb4[lv]
            for m0 in range(0, MT, 8):
                mc = min(8, MT - m0)
                fe4 = fsm.tile([128, 8, 128], BF16, tag="fe", name="fe4")
                mp = 128
                for m in range(m0, m0 + mc):
                    mp = min(128, HW - m * 128)
                    ps4 = ps2.tile([128, 4, HD], F32, tag="psA", name="ps4")
                    for ci, dr in enumerate((0, 1, W, W + 1)):
                        w0 = foff[lv] + m * 128 + dr
                        for k in range(2):
                            nc.tensor.matmul(
                                ps4[:mp, ci, :], lhsT=fsb[:, k, w0:w0 + mp],
                                rhs=ew[:, k, :], start=(k == 0), stop=(k == 1),
                            )
                    nc.scalar.copy(fe4[:mp, m - m0, :], ps4[:mp, :, :])
                nc.scalar.dma_start(
                    out=_ap(f4, m0 * 128 * 128,
                            [[128, mp], [128 * 128, mc], [1, 128]]),
                    in_=fe4[:mp, 0:mc, :],
                )

        def emit_logits(lv):
            # one-block software pipeline: the PE issues block g+1's
            # transposes while the ACT engine evacuates block g's xt, so the
            # PE never stalls on the cross-engine round-trip
            g0 = lv * BPL
            xqL = xpool.tile([128, BPL, 256], F32, tag="xq", name="xqL")
            nc.sync.dma_start(
                out=xqL[:],
                in_=_ap(x, lv * P * C, [[256, 128], [128 * 256, BPL], [1, 256]]),
            )
            pend = []

            def flush():
                gp_, xtp = pend.pop(0)
                lg = ps2.tile([128, 96], F32, tag="plg", name="lg")
                for k in range(2):
                    nc.tensor.matmul(lg[:], lhsT=xtp[:, k, :],
                                     rhs=wcat[:, k, :],
                                     start=(k == 0), stop=(k == 1))
                nc.scalar.copy(lgL[lv][:, gp_ - g0, :], lg[:])

            for g in range(g0, g0 + BPL):
                xt = blockio.tile([128, 2, 128], F32, tag="xt", name="xt")
                for k in range(2):
                    pt_ = ps.tile([128, 128], F32, tag="ptr", name="pt_")
                    nc.tensor.transpose(
                        pt_[:], xqL[:, g - g0, k * 128:(k + 1) * 128], ident[:])
                    nc.scalar.copy(xt[:, k, :], pt_[:])
                pend.append((g, xt))
                if len(pend) >= 2:
                    flush()
            while pend:
                flush()
            nc.vector.tensor_add(
                lgL[lv][:], lgL[lv][:],
                sv(bias96, 0, [[0, BPL], [1, 96]]))

        def emit_prep(lv):
            H, W = LEVEL_HW[lv]
            g0 = lv * BPL
            kap = 0.5 * (W - 1)
            ea = prep.tile([128, 256], F32, tag="ea", name="ea")
            nc.scalar.activation(
                ea[:], sv(lgL[lv], 0, [[96, BPL], [1, 32]]), AF.Exp)
            s2 = prep.tile([128, 128], F32, tag="s2", name="s2")
            nc.vector.tensor_add(s2[:], sv(ea, 0, [[4, 64], [1, 2]]),
                                 sv(ea, 2, [[4, 64], [1, 2]]))
            s1 = prep.tile([128, 64], F32, tag="s1", name="s1")
            nc.vector.tensor_add(s1[:], sv(s2, 0, [[2, 64]]),
                                 sv(s2, 1, [[2, 64]]))
            dinv = prep.tile([128, 64], F32, tag="dinv", name="dinv")
            nc.vector.reciprocal(dinv[:], s1[:])
            a_h = prep.tile([128, 256], F32, tag="a_h", name="a_h")
            nc.vector.tensor_mul(a_h[:], ea[:],
                                 sv(dinv, 0, [[1, 64], [0, 4]]))

            T1 = prep.tile([128, 512], F32, tag="T1", name="T1")
            nc.scalar.activation(
                T1[:], sv(lgL[lv], 32, [[96, BPL], [1, 64]]), AF.Tanh)
            nc.vector.tensor_add(T1[:], T1[:],
                                 sv(refc, g0 * 2, [[2, BPL], [0, 32], [1, 2]]))
            nc.vector.tensor_scalar(T1[:], T1[:], kap, kap, OP.mult, OP.add)
            # floor via int16 cast round-trip (all-DVE; the is_gt fix below
            # corrects any round-up, so truncate vs RNE both work)
            Tfi = prep.tile([128, 512], I16, tag="Tfi", name="Tfi")
            nc.vector.tensor_copy(Tfi[:], T1[:])
            T2 = prep.tile([128, 512], F32, tag="T2", name="T2")
            nc.vector.tensor_copy(T2[:], Tfi[:])
            T3 = prep.tile([128, 512], F32, tag="T3", name="T3")
            nc.vector.tensor_tensor(T3[:], T2[:], T1[:], OP.is_gt)
            nc.vector.tensor_tensor(T2[:], T2[:], T3[:], OP.subtract)   # x0f
            nc.vector.tensor_tensor(T3[:], T1[:], T2[:], OP.subtract)   # w1f
            nc.vector.tensor_scalar(T1[:], T3[:], -1.0, 1.0, OP.mult, OP.add)
            T4 = prep.tile([128, 512], F32, tag="T4", name="T4")  # xb
            nc.vector.tensor_scalar(T4[:], T2[:], 0.0, float(W - 2),
                                    OP.max, OP.min)
            nc.vector.tensor_tensor(T2[:], T2[:], T4[:], OP.subtract)   # d
            T5 = prep.tile([128, 512], F32, tag="T5", name="T5")  # e0 -> wB
            nc.vector.tensor_scalar(T5[:], T2[:], 0.0, None, OP.is_equal)
            T6 = prep.tile([128, 512], F32, tag="T6", name="T6")  # em1
            nc.vector.tensor_scalar(T6[:], T2[:], -1.0, None, OP.is_equal)
            nc.vector.tensor_scalar(T2[:], T2[:], 1.0, None, OP.is_equal)
            T7 = prep.tile([128, 512], F32, tag="T7", name="T7")  # wA
            nc.vector.tensor_tensor(T7[:], T1[:], T5[:], OP.mult)
            nc.vector.tensor_tensor(T6[:], T3[:], T6[:], OP.mult)
            nc.vector.tensor_add(T7[:], T7[:], T6[:])
            nc.vector.tensor_tensor(T5[:], T3[:], T5[:], OP.mult)
            nc.vector.tensor_tensor(T2[:], T1[:], T2[:], OP.mult)
            nc.vector.tensor_add(T5[:], T5[:], T2[:])

            fly = prep.tile([128, 256], F32, tag="fly", name="fly")
            nc.vector.tensor_scalar_mul(fly[:], sv(T4, 1, [[2, 256]]), float(W))
            nc.vector.tensor_add(fly[:], fly[:], sv(T4, 0, [[2, 256]]))
            T2i = prep.tile([128, 2, 128], I16, tag="T2i", name="T2i")
            for j in range(2):
                pf = ps.tile([128, 128], F32, tag="ptr", name="pf")
                nc.tensor.matmul(pf[:], lhsT=fly[:, j * 128:(j + 1) * 128],
                                 rhs=permP[:], start=True, stop=True)
                nc.vector.tensor_copy(T2i[:, j, :], pf[:])
            # bounce T2i through DRAM to land the wrapped index layout in 3
            # DMAs; gathers only use queues {0,1} so replication stops at
            # partition 64
            t2d = t2dL[lv]
            nc.sync.dma_start(out=_ap(t2d, 0, [[256, 128], [1, 256]]),
                              in_=T2i[:])
            for j in range(2):
                nc.sync.dma_start(
                    out=idxwL[lv][0:16, j * 1024:(j + 1) * 1024],
                    in_=_ap(t2d, j * 128, [[8, 16], [256, 128], [1, 8]]),
                )
            for t in range(1, 8):
                nc.sync.dma_start(
                    out=idxwL[lv][t * 16:(t + 1) * 16, :],
                    in_=idxwL[lv][0:16, :])

            # per-corner coefs; corner index = yi*2 + si to match the
            # [r, r+1, r+W, r+W+1] row layout of femb4.
            wxa = prep.tile([128, 256], F32, tag="wxa", name="wxa")
            nc.vector.tensor_mul(wxa[:], sv(T7, 0, [[2, 256]]), a_h[:])
            wxb = prep.tile([128, 256], F32, tag="wxb", name="wxb")
            nc.vector.tensor_mul(wxb[:], sv(T5, 0, [[2, 256]]), a_h[:])
            for si, wx in ((0, wxa), (1, wxb)):
                for yi, wy in ((0, T7), (1, T5)):
                    nc.vector.tensor_mul(
                        sv(c4L[lv], yi * 2 + si, [[4, 256]]),
                        wx[:],
                        sv(wy, 1, [[2, 256]]),
                    )

        # interleaved emission: each level's prep/T2i lands on the PE right
        # after that level's logits so its gathers unblock early; the heavy
        # femb matmul batches for the big levels go last (their gathers are
        # hundreds of us out)
        emit_logits(3)
        emit_prep(3)
        emit_femb(3)
        emit_femb(2)
        emit_logits(2)
        emit_prep(2)
        emit_logits(1)
        emit_prep(1)
        emit_femb(1)
        emit_logits(0)
        emit_prep(0)
        emit_femb(0)

        # ---- phase 3: gather + combine for all blocks ----
        for lv in LV_ORDER:
            H, W = LEVEL_HW[lv]
            HW = H * W
            g0 = lv * BPL
            for g in range(g0, g0 + BPL):
                gi = g - g0
                # expand coefs over the 32 head-dims on the Scalar engine
                c4x = cxp.tile([128, 4096], BF16, tag="c4x")
                nc.scalar.copy(
                    c4x[:],
                    sv(c4L[lv], gi * 128, [[4, 32], [1, 4], [0, 32]]),
                )
                gb = gpool.tile([128, 32, 128], BF16, tag="gb")
                for c in range(4):
                    nc.gpsimd.dma_gather(
                        gb[:, c * 8:(c + 1) * 8, :],
                        _ap(femb4[lv], 0, [[128, HW], [1, 128]]),
                        idxwL[lv][:, gi * 256 + c * 64:gi * 256 + (c + 1) * 64],
                        1024,
                        1024,
                        128,
                        elem_step=128,
                        queue_num=c,
                    )
                nc.vector.tensor_mul(gb[:], gb[:], c4x[:])
                t1 = tpool.tile([128, 2048], BF16, tag="t1")
                nc.vector.tensor_add(
                    t1[:],
                    sv(gb, 0, [[128, 32], [64, 2], [1, 32]]),
                    sv(gb, 32, [[128, 32], [64, 2], [1, 32]]),
                )
                t2 = tpool.tile([128, 1024], BF16, tag="t2")
                nc.vector.tensor_add(
                    t2[:],
                    sv(t1, 0, [[64, 32], [1, 32]]),
                    sv(t1, 32, [[64, 32], [1, 32]]),
                )
                t3 = tpool.tile([128, 512], BF16, tag="t3")
                nc.vector.tensor_add(
                    t3[:],
                    sv(t2, 0, [[128, 8], [1, 64]]),
                    sv(t2, 64, [[128, 8], [1, 64]]),
                )
                if gi % 4 == 0:
                    ob4 = opool.tile([128, 4, 256], F32, tag="ob", name="ob4")
                ob = ob4[:, gi % 4, :]
                nc.vector.tensor_add(
                    ob,
                    sv(t3, 0, [[64, 8], [1, 32]]),
                    sv(t3, 32, [[64, 8], [1, 32]]),
                )
                nc.vector.tensor_add(ob, ob,
                                     sv(ebt, lv * HD, [[0, 8], [1, 32]]))
                if gi % 4 == 3:
                    nc.sync.dma_start(
                        out=_ap(out, (g - 3) * 128 * 256,
                                [[256, 128], [128 * 256, 4], [1, 256]]),
                        in_=ob4[:],
                    )


def build_program():
    nc = bacc.Bacc("TRN2", target_bir_lowering=False, debug=False,
                   num_swdge_queues=4)
    io = {}
    io["x"] = nc.dram_tensor("x", [L, P, C], F32, kind="ExternalInput").ap()
    io["ref"] = nc.dram_tensor("ref", [L, P, 2], F32, kind="ExternalInput").ap()
    for i, (H, W) in enumerate(LEVEL_HW):
        io[f"feat{i}"] = nc.dram_tensor(f"feat{i}", [C, H, W], F32,
                                        kind="ExternalInput").ap()
    io["w_attn"] = nc.dram_tensor("w_attn", [C, NH * NS], F32,
                                  kind="ExternalInput").ap()
    io["b_attn"] = nc.dram_tensor("b_attn", [NH * NS], F32,
                                  kind="ExternalInput").ap()
    io["w_off"] = nc.dram_tensor("w_off", [C, 2 * NH * NS], F32,
                                 kind="ExternalInput").ap()
    io["b_off"] = nc.dram_tensor("b_off", [2 * NH * NS], F32,
                                 kind="ExternalInput").ap()
    io["embed_w"] = nc.dram_tensor("embed_w", [L, C, HD], F32,
                                   kind="ExternalInput").ap()
    io["embed_b"] = nc.dram_tensor("embed_b", [L, HD], F32,
                                   kind="ExternalInput").ap()
    io["permP"] = nc.dram_tensor("permP", [128, 128], F32,
                                 kind="ExternalInput").ap()
    io["out"] = nc.dram_tensor("out", [L, P, NH * HD], F32,
                               kind="ExternalOutput").ap()
    io["femb4"] = [
        nc.dram_tensor(f"femb4_{i}", [H * W, 128], BF16, kind="Internal").ap()
        for i, (H, W) in enumerate(LEVEL_HW)
    ]
    io["t2d"] = [
        nc.dram_tensor(f"t2d_{i}", [128 * 256], I16, kind="Internal").ap()
        for i in range(L)
    ]
    with tile.TileContext(nc) as tc:
        with ExitStack() as ctx:
            emit_kernel(ctx, tc, io)
    nc.compile()
    return nc


_prog = None


def kernel(**inputs):
    global _prog
    if _prog is None:
        _prog = build_program()
    nc = _prog
    res = run_bass_kernel_spmd(nc, _in_maps(inputs), list(range(B)))
    out = np.stack([res.results[i]["out"] for i in range(B)], axis=0)
    return out.reshape(B, L, P, NH * HD)


def _perm_matrix():
    p = np.zeros((128, 128), np.float32)
    for n in range(128):
        p[(n % 8) * 16 + n // 8, n] = 1.0
    return p


def _in_maps(inputs):
    keys = ["x", "ref", "feat0", "feat1", "feat2", "feat3",
            "w_attn", "b_attn", "w_off", "b_off", "embed_w", "embed_b"]
    per_batch = {"x", "ref", "feat0", "feat1", "feat2", "feat3"}
    pm = _perm_matrix()
    maps = []
    for b in range(B):
        m = {"permP": pm}
        for kk in keys:
            v = np.ascontiguousarray(np.asarray(inputs[kk], dtype=np.float32))
            m[kk] = v[b] if kk in per_batch else v
        maps.append(m)
    return maps


def profile(inputs):
    """Run with tracing; returns HW exec time in ns (or None if unavailable)."""
    global _prog
    if _prog is None:
        _prog = build_program()
    res = run_bass_kernel_spmd(_prog, _in_maps(inputs), list(range(B)), trace=True)
    return res.exec_time_ns


if __name__ == "__main__":
    build_program()
    print("build ok")
